# revision 27
# baseline (speedup 1.0000x reference)
"""Trainium2 Bass kernel for OldNeighborhoodEncoder (segment_reduce).

Math (reference):
    fc1    = relu(X @ W1.T + b1)            # [N, 64], X = [N, 3]
    pooled = segment_max(fc1, cluster, S)   # [S, 64], cluster = arange(N)//32
    h      = relu(pooled @ W1g.T + b1g)     # [S, 64]
    out    = relu(h @ W2g.T + b2g)          # [S, 128]

Hardcoded sizes: N=1048576, S=32768 (32 pts/cluster), 8 cores; core d does
points [d*131072, (d+1)*131072) == clusters [d*4096, (d+1)*4096).

v3 design (per core). Measured HW facts this is built around: PE is pinned
at 1.2 GHz (no p-state ramp; 512-col matmul = 427ns, exactly FD*0.833ns),
DVE tensor_reduce from PSUM = 1 elem/cyc/lane @0.96GHz, ACT activation =
1x @1.2GHz, DVE tensor_max on SBUF bf16 hits 2x_1P (2 results/cyc).

  - bf16 everywhere except PSUM/bias/output. xt [6, 65536]: col c = 512g+o;
    rows 0-2 = xyz of point 1024g+o, rows 3-5 = xyz of point 1024g+512+o;
    wpack [6,128] = blockdiag(W1.T x2) -> one col = fc1 of TWO points.
  - 43 psum chunks of 3 banks ([128, 3, 16, 32]; last 2 banks). Drains are
    split across the two PSUM read ports to keep pace with PE:
      A-chunks (31): ACT relu(+b1)-copies PSUM->SBUF bf16 (~1.66us);
        DVE later runs a tensor_max tree on the copy at 2x. Trees are
        DEFERRED (emitted 1+ chunks later) and BATCHED in pairs (L1-L4
        over 3072 cols, two per-chunk L5s) so DVE ops never gate the
        psum ping-pong or the ACT copy cadence.
      D-chunks (12): DVE reduce_max straight from PSUM + a cheap 4x
        tensor_scalar relu(+b1) on the [128,48] pooled slice.
    SBUF copies land in a 4-slot rotating buffer (one big tile) so pair
    batching works regardless of interleaving.
  - Tail MLP pipelined into the loop in slices (512,512,512,384,128 cols;
    stage1 at blocks 11/22/32/40/end, stage2 two blocks later), two
    dedicated psum banks so mm2A/mm2B run back-to-back; all tail relus
    on ACT (DVE is the fuller engine); slice DMAs stripe over all 16 hw
    queues as they complete. Only the 128-col slice trails the loop.
  - PSUM: 2x3 banks main + 2 tail = 8.
  - Other measured walls: both-PSUM tensor_tensor is illegal (NCC_IBVF027,
    one PSUM operand max), TRN2 matmul cannot emit bf16 PSUM, per-op init
    costs (ACT ~310cyc, DVE 60-120cyc) make whole-chunk drain
    specialization beat per-chunk ACT/DVE splitting.
    HW exec ~91.3us vs 104-115us baseline.
"""

import sys
import numpy as np
import ml_dtypes

if "/opt/trn_rl_repo" not in sys.path:
    sys.path.insert(0, "/opt/trn_rl_repo")

BF16 = ml_dtypes.bfloat16

N = 1048576
S = 32768
PTS_PER_CLUSTER = 32
FEATURE = 64
FG0 = 64
FG1 = 128
NCORES = 8
NPC = N // NCORES          # 131072 points per core
SPC = S // NCORES          # 4096 clusters per core
G = NPC // 1024            # 128 column-groups of 512

NCHUNKS = 43
BANKS = [3] * 42 + [2]
# whole-chunk drain specialization (minimizes per-op overheads):
# A-chunks: ACT relu-copies all psum cols to SBUF bf16 for a deferred,
# pair-batched DVE tensor_max tree. D-chunks: DVE reduce_max from PSUM.
D_CHUNKS = {2, 5, 8, 13, 16, 19, 24, 27, 30, 34, 36, 38}
SLOTW = 1536
# block -> (tail slice id, col lo, col hi); stage1 emitted at block key,
# stage2 two blocks later (inline at the end for the last slice)
TAIL_SLICES = {11: (0, 0, 512), 22: (1, 512, 1024), 32: (2, 1024, 1536),
               40: (3, 1536, 1920)}
FLUSH_AT = {10, 21, 31, 39}   # pop all queued trees before these stage1s

_PROGRAM = None


def _build_program():
    from concourse import bacc, bass, tile

    mybir = bass.mybir
    f32 = mybir.dt.float32
    bf16 = mybir.dt.bfloat16
    AX = mybir.AxisListType
    add = mybir.AluOpType.add
    vmax = mybir.AluOpType.max
    Relu = mybir.ActivationFunctionType.Relu

    nc = bacc.Bacc("TRN2", target_bir_lowering=False, debug=False)

    xt = nc.dram_tensor("xt", [6, G * 512], bf16, kind="ExternalInput").ap()
    wpack = nc.dram_tensor("wpack", [6, 128], bf16, kind="ExternalInput").ap()
    b1d = nc.dram_tensor("b1d", [128, 1], f32, kind="ExternalInput").ap()
    w1gbd = nc.dram_tensor("w1gbd", [128, 128], bf16, kind="ExternalInput").ap()
    b1gd = nc.dram_tensor("b1gd", [128, 1], f32, kind="ExternalInput").ap()
    w2gt = nc.dram_tensor("w2gt", [128, 128], bf16, kind="ExternalInput").ap()
    b2g = nc.dram_tensor("b2g", [128, 1], f32, kind="ExternalInput").ap()
    outA = nc.dram_tensor("outA", [128, 2048], f32, kind="ExternalOutput").ap()
    outB = nc.dram_tensor("outB", [128, 2048], f32, kind="ExternalOutput").ap()

    with tile.TileContext(nc) as tc:
        with (
            tc.tile_pool(name="w", bufs=1) as wp,
            tc.tile_pool(name="x", bufs=6) as xp,
            tc.tile_pool(name="tr", bufs=2) as trp,
            tc.tile_pool(name="pd", bufs=2) as pdp,
            tc.tile_pool(name="hr", bufs=2) as hrp,
            tc.tile_pool(name="acc", bufs=1) as accp,
            tc.tile_pool(name="ps", bufs=2, space=bass.MemorySpace.PSUM) as pp,
            tc.tile_pool(name="tpa", bufs=1, space=bass.MemorySpace.PSUM) as tpa,
            tc.tile_pool(name="tpb", bufs=1, space=bass.MemorySpace.PSUM) as tpb,
        ):
            wpack_t = wp.tile([6, 128], bf16, tag="wpack")
            b1d_t = wp.tile([128, 1], f32, tag="b1d")
            w1gbd_t = wp.tile([128, 128], bf16, tag="w1gbd")
            b1gd_t = wp.tile([128, 1], f32, tag="b1gd")
            w2gt_t = wp.tile([128, 128], bf16, tag="w2gt")
            b2g_t = wp.tile([128, 1], f32, tag="b2g")
            for t, d in (
                (wpack_t, wpack),
                (b1d_t, b1d),
                (w1gbd_t, w1gbd),
                (b1gd_t, b1gd),
                (w2gt_t, w2gt),
                (b2g_t, b2g),
            ):
                nc.scalar.dma_start(t[:], d[:])

            pooledR = accp.tile([128, 2048], bf16, tag="pooledR")
            o2A = accp.tile([128, 2048], f32, tag="o2A")
            o2B = accp.tile([128, 2048], f32, tag="o2B")
            # 4-slot rotating buffer for ACT's relu-copies (one tile so
            # adjacent slots can be tree-reduced in one batched op)
            sbbig = accp.tile([128, 4, SLOTW], bf16, tag="sbbig")

            tpsA = tpa.tile([128, 512], f32, tag="tpsA")
            tpsB = tpb.tile([128, 512], f32, tag="tpsB")

            tail_hr = {}

            def tail_stage1(j, lo, hi):
                w = hi - lo
                nc.tensor.matmul(tpsA[:, 0:w], w1gbd_t[:], pooledR[:, lo:hi])
                hR = hrp.tile([128, 512], bf16, tag="hR")
                nc.scalar.activation(hR[:, 0:w], tpsA[:, 0:w], Relu,
                                     bias=b1gd_t[:])
                tail_hr[j] = hR

            def tail_stage2(j, lo, hi):
                w = hi - lo
                hR = tail_hr.pop(j)
                nc.tensor.matmul(tpsA[:, 0:w], w2gt_t[0:64, :], hR[0:64, 0:w])
                nc.tensor.matmul(tpsB[:, 0:w], w2gt_t[64:128, :],
                                 hR[64:128, 0:w])
                nc.scalar.activation(o2A[:, lo:hi], tpsA[:, 0:w], Relu,
                                     bias=b2g_t[:])
                nc.scalar.activation(o2B[:, lo:hi], tpsB[:, 0:w], Relu,
                                     bias=b2g_t[:])
                nc.sync.dma_start(outA[:, lo:hi], o2A[:, lo:hi])
                nc.sync.dma_start(outB[:, lo:hi], o2B[:, lo:hi])

            # deferred tree machinery
            pending = []      # [(chunk, slot, cols)] copies not yet treed
            tree_q = []       # emission thunks, popped one per block
            a_count = 0

            def emit_pair_tree(c1, s1, c2, s2):
                # batched L1-L4 over both slots (contiguous), per-chunk L5
                rr = 2 * (SLOTW // 32)
                half = SLOTW // 32
                v = sbbig[:, s1 : s1 + 2].rearrange("p s (r t) -> p (s r) t",
                                                    t=32)
                y1 = trp.tile([128, rr, 16], bf16, tag="y1")
                y2 = trp.tile([128, rr, 8], bf16, tag="y2")
                y3 = trp.tile([128, rr, 4], bf16, tag="y3")
                y4 = trp.tile([128, rr, 2], bf16, tag="y4")
                nc.vector.tensor_max(y1[:], v[:, :, 0:16], v[:, :, 16:32])
                nc.vector.tensor_max(y2[:], y1[:, :, 0:8], y1[:, :, 8:16])
                nc.vector.tensor_max(y3[:], y2[:, :, 0:4], y2[:, :, 4:8])
                nc.vector.tensor_max(y4[:], y3[:, :, 0:2], y3[:, :, 2:4])
                if c2 == c1 + 1:
                    # adjacent pooled ranges -> single merged L5
                    p0 = 48 * c1
                    nc.vector.tensor_max(
                        pooledR[:, p0 : p0 + 2 * half],
                        y4[:, 0 : 2 * half, 0],
                        y4[:, 0 : 2 * half, 1],
                    )
                else:
                    for c, off in ((c1, 0), (c2, half)):
                        p0 = 48 * c
                        nc.vector.tensor_max(
                            pooledR[:, p0 : p0 + half],
                            y4[:, off : off + half, 0],
                            y4[:, off : off + half, 1],
                        )

            def emit_single_tree(c, slot, cols):
                r = cols // 32
                v = sbbig[:, slot, 0:cols].rearrange("p (r t) -> p r t", t=32)
                y1 = trp.tile([128, 72, 16], bf16, tag="ys1")
                y2 = trp.tile([128, 72, 8], bf16, tag="ys2")
                y3 = trp.tile([128, 72, 4], bf16, tag="ys3")
                y4 = trp.tile([128, 72, 2], bf16, tag="ys4")
                nc.vector.tensor_max(y1[:, 0:r], v[:, :, 0:16], v[:, :, 16:32])
                nc.vector.tensor_max(y2[:, 0:r], y1[:, 0:r, 0:8],
                                     y1[:, 0:r, 8:16])
                nc.vector.tensor_max(y3[:, 0:r], y2[:, 0:r, 0:4],
                                     y2[:, 0:r, 4:8])
                nc.vector.tensor_max(y4[:, 0:r], y3[:, 0:r, 0:2],
                                     y3[:, 0:r, 2:4])
                p0 = 48 * c
                nc.vector.tensor_max(
                    pooledR[:, p0 : p0 + r],
                    y4[:, 0:r, 0],
                    y4[:, 0:r, 1],
                )

            def queue_trees(force=False):
                while len(pending) >= 2:
                    (c1, s1, w1), (c2, s2, w2) = pending[0], pending[1]
                    if (s1 % 2 == 0 and s2 == s1 + 1
                            and w1 == SLOTW and w2 == SLOTW):
                        tree_q.append(lambda a=c1, b=s1, c=c2, d=s2:
                                      emit_pair_tree(a, b, c, d))
                        del pending[0:2]
                    else:
                        tree_q.append(lambda a=c1, b=s1, w=w1:
                                      emit_single_tree(a, b, w))
                        del pending[0]
                if force and pending:
                    c1, s1, w1 = pending.pop(0)
                    tree_q.append(lambda a=c1, b=s1, w=w1:
                                  emit_single_tree(a, b, w))

            for c in range(NCHUNKS):
                nb = BANKS[c]
                cols = 512 * nb
                x0 = 1536 * c
                p0 = 48 * c

                xt_t = xp.tile([6, 1536], bf16, tag="xt")
                if c == 0:
                    nc.sync.dma_start(xt_t[:, 0:512], xt[:, 0:512])
                    nc.sync.dma_start(xt_t[:, 512:cols], xt[:, 512:cols])
                else:
                    nc.sync.dma_start(xt_t[:, 0:cols], xt[:, x0 : x0 + cols])

                ps = pp.tile([128, 3, 16, 32], f32, tag="ps")
                for b in range(nb):
                    nc.tensor.matmul(
                        ps[:, b], wpack_t[:], xt_t[:, 512 * b : 512 * (b + 1)]
                    )

                # stage2 of the tail slice started two blocks ago
                for blk, (j, lo, hi) in TAIL_SLICES.items():
                    if c == blk + 2:
                        tail_stage2(j, lo, hi)

                if c in D_CHUNKS:
                    pc = 16 * nb
                    pd = pdp.tile([128, 48], f32, tag="pd")
                    nc.vector.reduce_max(
                        pd[:, 0:pc].rearrange("p (b q) -> p b q", b=nb),
                        ps[:, 0:nb],
                        axis=AX.X,
                    )
                    nc.vector.tensor_scalar(
                        pooledR[:, p0 : p0 + pc], pd[:, 0:pc], b1d_t[:], 0.0,
                        op0=add, op1=vmax,
                    )
                else:
                    slot = a_count % 4
                    a_count += 1
                    nc.scalar.activation(
                        sbbig[:, slot, 0:cols], ps[:, 0:nb], Relu,
                        bias=b1d_t[:],
                    )
                    pending.append((c, slot, cols))
                    queue_trees()

                if c in FLUSH_AT:
                    queue_trees(force=True)
                    while tree_q:
                        tree_q.pop(0)()
                    a_count = (a_count + 1) // 2 * 2  # keep pairs forming
                elif tree_q:
                    tree_q.pop(0)()

                if c in TAIL_SLICES:
                    tail_stage1(*TAIL_SLICES[c])

            # final chunk's tree + last tail slice (cols 1920:2048)
            queue_trees(force=True)
            while tree_q:
                tree_q.pop(0)()
            tail_stage1(4, 1920, 2048)
            tail_stage2(4, 1920, 2048)

    nc.compile()
    return nc


def _get_program():
    global _PROGRAM
    if _PROGRAM is None:
        _PROGRAM = _build_program()
    return _PROGRAM


def _host_pack(relative_points, W1, b1, W1g, b1g, W2g, b2g):
    X = np.ascontiguousarray(relative_points, dtype=np.float32)
    W1 = np.asarray(W1, np.float32)
    b1 = np.asarray(b1, np.float32)
    W1g = np.asarray(W1g, np.float32)
    b1g = np.asarray(b1g, np.float32)
    W2g = np.asarray(W2g, np.float32)
    b2g = np.asarray(b2g, np.float32)

    wpack = np.zeros((6, 128), np.float32)
    wpack[0:3, 0:64] = W1.T
    wpack[3:6, 64:128] = W1.T
    b1d = np.concatenate([b1, b1]).reshape(128, 1)
    w1gbd = np.zeros((128, 128), np.float32)
    w1gbd[0:64, 0:64] = W1g.T
    w1gbd[64:128, 64:128] = W1g.T
    b1gd = np.concatenate([b1g, b1g]).reshape(128, 1)
    w2gt = np.vstack([W2g.T, W2g.T])  # [128, 128]
    b2gc = np.ascontiguousarray(b2g.reshape(128, 1))

    wpack = wpack.astype(BF16)
    w1gbd = w1gbd.astype(BF16)
    w2gt = np.ascontiguousarray(w2gt.astype(BF16))

    in_maps = []
    for d in range(NCORES):
        Xc = X[d * NPC : (d + 1) * NPC]
        xt6 = np.ascontiguousarray(
            Xc.reshape(G, 2, 512, 3)
            .transpose(1, 3, 0, 2)
            .reshape(6, G * 512)
            .astype(BF16)
        )
        in_maps.append(
            {
                "xt": xt6,
                "wpack": wpack,
                "b1d": b1d,
                "w1gbd": w1gbd,
                "b1gd": b1gd,
                "w2gt": w2gt,
                "b2g": b2gc,
            }
        )
    return in_maps


def _host_unpack(results):
    out = np.empty((S, FG1), np.float32)
    for d in range(NCORES):
        oA = results[d]["outA"].reshape(128, 32, 4, 16)
        oB = results[d]["outB"].reshape(128, 32, 4, 16)
        blk = out[d * SPC : (d + 1) * SPC].reshape(32, 4, 2, 16, 128)
        blk[:, :, 0] = oA.transpose(1, 2, 3, 0)
        blk[:, :, 1] = oB.transpose(1, 2, 3, 0)
    return out


def _numpy_fallback(relative_points, cluster, num_clusters,
                    W1, b1, W1g, b1g, W2g, b2g):
    X = np.asarray(relative_points, np.float32)
    fc1 = np.maximum(X @ np.asarray(W1, np.float32).T + np.asarray(b1, np.float32), 0.0)
    Sn = int(num_clusters)
    cl = np.asarray(cluster).astype(np.int64)
    pooled = np.full((Sn, fc1.shape[1]), -np.inf, np.float32)
    starts = np.flatnonzero(np.r_[True, cl[1:] != cl[:-1]])
    seg_ids = cl[starts]
    pooled[seg_ids] = np.maximum.reduceat(fc1, starts, axis=0)
    h = np.maximum(pooled @ np.asarray(W1g, np.float32).T + np.asarray(b1g, np.float32), 0.0)
    return np.maximum(h @ np.asarray(W2g, np.float32).T + np.asarray(b2g, np.float32), 0.0).astype(np.float32)


def _run_hw(in_maps, trace=False):
    from concourse.bass_utils import run_bass_kernel_spmd

    nc = _get_program()
    return run_bass_kernel_spmd(
        nc, in_maps, list(range(NCORES)), trace=trace
    )


def kernel(relative_points, cluster, num_clusters,
           W1, b1, W1g, b1g, W2g, b2g):
    cl = np.asarray(cluster)
    expected_cl = np.arange(N, dtype=np.int64) // PTS_PER_CLUSTER
    if (
        relative_points.shape != (N, 3)
        or int(num_clusters) != S
        or not np.array_equal(cl, expected_cl)
    ):
        return _numpy_fallback(relative_points, cluster, num_clusters,
                               W1, b1, W1g, b1g, W2g, b2g)

    in_maps = _host_pack(relative_points, W1, b1, W1g, b1g, W2g, b2g)
    res = _run_hw(in_maps, trace=False)
    return _host_unpack(res.results)


def run_traced(inputs):
    """test.py helper: returns (output, exec_time_ns)."""
    in_maps = _host_pack(
        inputs["relative_points"], inputs["W1"], inputs["b1"],
        inputs["W1g"], inputs["b1g"], inputs["W2g"], inputs["b2g"],
    )
    res = _run_hw(in_maps, trace=True)
    return _host_unpack(res.results), res.exec_time_ns


# revision 28
# speedup vs baseline: 1.2202x; 1.2202x over previous
"""Trainium2 Bass kernel for OldNeighborhoodEncoder (segment_reduce).

Math (reference):
    fc1    = relu(X @ W1.T + b1)            # [N, 64], X = [N, 3]
    pooled = segment_max(fc1, cluster, S)   # [S, 64], cluster = arange(N)//32
    h      = relu(pooled @ W1g.T + b1g)     # [S, 64]
    out    = relu(h @ W2g.T + b2g)          # [S, 128]

Hardcoded sizes: N=1048576, S=32768 (32 pts/cluster), 8 cores; core d does
points [d*131072, (d+1)*131072) == clusters [d*4096, (d+1)*4096).

v3 design (per core). Measured HW facts this is built around: PE is pinned
at 1.2 GHz (no p-state ramp; 512-col matmul = 427ns, exactly FD*0.833ns),
DVE tensor_reduce from PSUM = 1 elem/cyc/lane @0.96GHz, ACT activation =
1x @1.2GHz, DVE tensor_max on SBUF bf16 hits 2x_1P (2 results/cyc).

  - bf16 everywhere except PSUM/bias/output. xt [6, 65536]: col c = 512g+o;
    rows 0-2 = xyz of point 1024g+o, rows 3-5 = xyz of point 1024g+512+o;
    wpack [6,128] = blockdiag(W1.T x2) -> one col = fc1 of TWO points.
  - 43 psum chunks of 3 banks ([128, 3, 16, 32]; last 2 banks). Drains are
    split across the two PSUM read ports to keep pace with PE:
      A-chunks (31): ACT relu(+b1)-copies PSUM->SBUF bf16 (~1.66us);
        DVE later runs a tensor_max tree on the copy at 2x. Trees are
        DEFERRED (emitted 1+ chunks later) and BATCHED in pairs (L1-L4
        over 3072 cols, two per-chunk L5s) so DVE ops never gate the
        psum ping-pong or the ACT copy cadence.
      D-chunks (12): DVE reduce_max straight from PSUM + a cheap 4x
        tensor_scalar relu(+b1) on the [128,48] pooled slice.
    SBUF copies land in a 4-slot rotating buffer (one big tile) so pair
    batching works regardless of interleaving.
  - Tail MLP pipelined into the loop in slices (512,512,512,384,128 cols;
    stage1 at blocks 11/22/32/40/end, stage2 two blocks later), two
    dedicated psum banks so mm2A/mm2B run back-to-back; all tail relus
    on ACT (DVE is the fuller engine); slice DMAs stripe over all 16 hw
    queues as they complete. Only the 128-col slice trails the loop.
  - PSUM: 2x3 banks main + 2 tail = 8.
  - Other measured walls: both-PSUM tensor_tensor is illegal (NCC_IBVF027,
    one PSUM operand max), TRN2 matmul cannot emit bf16 PSUM, per-op init
    costs (ACT ~310cyc, DVE 60-120cyc) make whole-chunk drain
    specialization beat per-chunk ACT/DVE splitting.
    HW exec ~91.3us vs 104-115us baseline.
"""

import sys
import numpy as np
import ml_dtypes

if "/opt/trn_rl_repo" not in sys.path:
    sys.path.insert(0, "/opt/trn_rl_repo")

BF16 = ml_dtypes.bfloat16

N = 1048576
S = 32768
PTS_PER_CLUSTER = 32
FEATURE = 64
FG0 = 64
FG1 = 128
NCORES = 8
NPC = N // NCORES          # 131072 points per core
SPC = S // NCORES          # 4096 clusters per core
G = NPC // 1024            # 128 column-groups of 512

NCHUNKS = 43
BANKS = [3] * 42 + [2]
# whole-chunk drain specialization (minimizes per-op overheads):
# A-chunks: ACT relu-copies all psum cols to SBUF bf16 for a deferred,
# pair-batched DVE tensor_max tree. D-chunks: DVE reduce_max from PSUM.
D_CHUNKS = {3, 6, 9, 14, 17, 20, 25, 28, 33, 36, 39, 41}
SLOTW = 1536
# block -> (tail slice id, col lo, col hi); stage1 emitted at block key,
# stage2 two blocks later (inline at the end for the last slice)
TAIL_SLICES = {11: (0, 0, 512), 22: (1, 512, 1024), 32: (2, 1024, 1536),
               40: (3, 1536, 1920)}
FLUSH_AT = {10, 21, 31, 39}   # pop all queued trees before these stage1s

_PROGRAM = None


def _build_program():
    from concourse import bacc, bass, tile

    mybir = bass.mybir
    f32 = mybir.dt.float32
    bf16 = mybir.dt.bfloat16
    AX = mybir.AxisListType
    add = mybir.AluOpType.add
    vmax = mybir.AluOpType.max
    Relu = mybir.ActivationFunctionType.Relu

    nc = bacc.Bacc("TRN2", target_bir_lowering=False, debug=False)

    xt = nc.dram_tensor("xt", [6, G * 512], bf16, kind="ExternalInput").ap()
    wpack = nc.dram_tensor("wpack", [6, 128], bf16, kind="ExternalInput").ap()
    b1d = nc.dram_tensor("b1d", [128, 1], f32, kind="ExternalInput").ap()
    w1gbd = nc.dram_tensor("w1gbd", [128, 128], bf16, kind="ExternalInput").ap()
    b1gd = nc.dram_tensor("b1gd", [128, 1], f32, kind="ExternalInput").ap()
    w2gt = nc.dram_tensor("w2gt", [128, 128], bf16, kind="ExternalInput").ap()
    b2g = nc.dram_tensor("b2g", [128, 1], f32, kind="ExternalInput").ap()
    outA = nc.dram_tensor("outA", [128, 2048], f32, kind="ExternalOutput").ap()
    outB = nc.dram_tensor("outB", [128, 2048], f32, kind="ExternalOutput").ap()

    with tile.TileContext(nc) as tc:
        with (
            tc.tile_pool(name="w", bufs=1) as wp,
            tc.tile_pool(name="x", bufs=6) as xp,
            tc.tile_pool(name="tr", bufs=2) as trp,
            tc.tile_pool(name="pd", bufs=2) as pdp,
            tc.tile_pool(name="hr", bufs=2) as hrp,
            tc.tile_pool(name="acc", bufs=1) as accp,
            tc.tile_pool(name="ps", bufs=2, space=bass.MemorySpace.PSUM) as pp,
            tc.tile_pool(name="tpa", bufs=1, space=bass.MemorySpace.PSUM) as tpa,
            tc.tile_pool(name="tpb", bufs=1, space=bass.MemorySpace.PSUM) as tpb,
        ):
            wpack_t = wp.tile([6, 128], bf16, tag="wpack")
            b1d_t = wp.tile([128, 1], f32, tag="b1d")
            w1gbd_t = wp.tile([128, 128], bf16, tag="w1gbd")
            b1gd_t = wp.tile([128, 1], f32, tag="b1gd")
            w2gt_t = wp.tile([128, 128], bf16, tag="w2gt")
            b2g_t = wp.tile([128, 1], f32, tag="b2g")
            for t, d in (
                (wpack_t, wpack),
                (b1d_t, b1d),
                (w1gbd_t, w1gbd),
                (b1gd_t, b1gd),
                (w2gt_t, w2gt),
                (b2g_t, b2g),
            ):
                nc.scalar.dma_start(t[:], d[:])

            pooledR = accp.tile([128, 2048], bf16, tag="pooledR")
            o2A = accp.tile([128, 2048], f32, tag="o2A")
            o2B = accp.tile([128, 2048], f32, tag="o2B")
            # 4-slot rotating buffer for ACT's relu-copies (one tile so
            # adjacent slots can be tree-reduced in one batched op)
            sbbig = accp.tile([128, 4, SLOTW], bf16, tag="sbbig")

            tpsA = tpa.tile([128, 512], f32, tag="tpsA")
            tpsB = tpb.tile([128, 512], f32, tag="tpsB")

            tail_hr = {}

            def tail_stage1(j, lo, hi):
                w = hi - lo
                nc.tensor.matmul(tpsA[:, 0:w], w1gbd_t[:], pooledR[:, lo:hi])
                hR = hrp.tile([128, 512], bf16, tag="hR")
                nc.scalar.activation(hR[:, 0:w], tpsA[:, 0:w], Relu,
                                     bias=b1gd_t[:])
                tail_hr[j] = hR

            def tail_stage2(j, lo, hi):
                w = hi - lo
                hR = tail_hr.pop(j)
                nc.tensor.matmul(tpsA[:, 0:w], w2gt_t[0:64, :], hR[0:64, 0:w])
                nc.tensor.matmul(tpsB[:, 0:w], w2gt_t[64:128, :],
                                 hR[64:128, 0:w])
                nc.scalar.activation(o2A[:, lo:hi], tpsA[:, 0:w], Relu,
                                     bias=b2g_t[:])
                nc.scalar.activation(o2B[:, lo:hi], tpsB[:, 0:w], Relu,
                                     bias=b2g_t[:])
                nc.sync.dma_start(outA[:, lo:hi], o2A[:, lo:hi])
                nc.sync.dma_start(outB[:, lo:hi], o2B[:, lo:hi])

            # deferred tree machinery
            pending = []      # [(chunk, slot, cols)] copies not yet treed
            tree_q = []       # emission thunks, popped one per block
            a_count = 0

            def emit_pair_tree(c1, s1, c2, s2):
                # batched L1-L4 over both slots (contiguous), per-chunk L5
                rr = 2 * (SLOTW // 32)
                half = SLOTW // 32
                v = sbbig[:, s1 : s1 + 2].rearrange("p s (r t) -> p (s r) t",
                                                    t=32)
                y1 = trp.tile([128, rr, 16], bf16, tag="y1")
                y2 = trp.tile([128, rr, 8], bf16, tag="y2")
                y3 = trp.tile([128, rr, 4], bf16, tag="y3")
                y4 = trp.tile([128, rr, 2], bf16, tag="y4")
                nc.vector.tensor_max(y1[:], v[:, :, 0:16], v[:, :, 16:32])
                nc.vector.tensor_max(y2[:], y1[:, :, 0:8], y1[:, :, 8:16])
                nc.vector.tensor_max(y3[:], y2[:, :, 0:4], y2[:, :, 4:8])
                nc.vector.tensor_max(y4[:], y3[:, :, 0:2], y3[:, :, 2:4])
                if c2 == c1 + 1:
                    # adjacent pooled ranges -> single merged L5
                    p0 = 48 * c1
                    nc.vector.tensor_max(
                        pooledR[:, p0 : p0 + 2 * half],
                        y4[:, 0 : 2 * half, 0],
                        y4[:, 0 : 2 * half, 1],
                    )
                else:
                    for c, off in ((c1, 0), (c2, half)):
                        p0 = 48 * c
                        nc.vector.tensor_max(
                            pooledR[:, p0 : p0 + half],
                            y4[:, off : off + half, 0],
                            y4[:, off : off + half, 1],
                        )

            def emit_single_tree(c, slot, cols):
                r = cols // 32
                v = sbbig[:, slot, 0:cols].rearrange("p (r t) -> p r t", t=32)
                y1 = trp.tile([128, 72, 16], bf16, tag="ys1")
                y2 = trp.tile([128, 72, 8], bf16, tag="ys2")
                y3 = trp.tile([128, 72, 4], bf16, tag="ys3")
                y4 = trp.tile([128, 72, 2], bf16, tag="ys4")
                nc.vector.tensor_max(y1[:, 0:r], v[:, :, 0:16], v[:, :, 16:32])
                nc.vector.tensor_max(y2[:, 0:r], y1[:, 0:r, 0:8],
                                     y1[:, 0:r, 8:16])
                nc.vector.tensor_max(y3[:, 0:r], y2[:, 0:r, 0:4],
                                     y2[:, 0:r, 4:8])
                nc.vector.tensor_max(y4[:, 0:r], y3[:, 0:r, 0:2],
                                     y3[:, 0:r, 2:4])
                p0 = 48 * c
                nc.vector.tensor_max(
                    pooledR[:, p0 : p0 + r],
                    y4[:, 0:r, 0],
                    y4[:, 0:r, 1],
                )

            def queue_trees(force=False):
                while len(pending) >= 2:
                    (c1, s1, w1), (c2, s2, w2) = pending[0], pending[1]
                    if (s1 % 2 == 0 and s2 == s1 + 1
                            and w1 == SLOTW and w2 == SLOTW):
                        tree_q.append(lambda a=c1, b=s1, c=c2, d=s2:
                                      emit_pair_tree(a, b, c, d))
                        del pending[0:2]
                    else:
                        tree_q.append(lambda a=c1, b=s1, w=w1:
                                      emit_single_tree(a, b, w))
                        del pending[0]
                if force and pending:
                    c1, s1, w1 = pending.pop(0)
                    tree_q.append(lambda a=c1, b=s1, w=w1:
                                  emit_single_tree(a, b, w))

            for c in range(NCHUNKS):
                nb = BANKS[c]
                cols = 512 * nb
                x0 = 1536 * c
                p0 = 48 * c

                xt_t = xp.tile([6, 1536], bf16, tag="xt")
                if c == 0:
                    nc.sync.dma_start(xt_t[:, 0:512], xt[:, 0:512])
                    nc.sync.dma_start(xt_t[:, 512:cols], xt[:, 512:cols])
                else:
                    nc.sync.dma_start(xt_t[:, 0:cols], xt[:, x0 : x0 + cols])

                ps = pp.tile([128, 3, 16, 32], f32, tag="ps")
                for b in range(nb):
                    nc.tensor.matmul(
                        ps[:, b], wpack_t[:], xt_t[:, 512 * b : 512 * (b + 1)]
                    )

                # stage2 of the tail slice started two blocks ago
                for blk, (j, lo, hi) in TAIL_SLICES.items():
                    if c == blk + 2:
                        tail_stage2(j, lo, hi)

                if c in D_CHUNKS:
                    pc = 16 * nb
                    pd = pdp.tile([128, 48], f32, tag="pd")
                    nc.vector.reduce_max(
                        pd[:, 0:pc].rearrange("p (b q) -> p b q", b=nb),
                        ps[:, 0:nb],
                        axis=AX.X,
                    )
                    nc.vector.tensor_scalar(
                        pooledR[:, p0 : p0 + pc], pd[:, 0:pc], b1d_t[:], 0.0,
                        op0=add, op1=vmax,
                    )
                else:
                    slot = a_count % 4
                    a_count += 1
                    nc.scalar.activation(
                        sbbig[:, slot, 0:cols], ps[:, 0:nb], Relu,
                        bias=b1d_t[:],
                    )
                    pending.append((c, slot, cols))
                    queue_trees()

                if c in FLUSH_AT:
                    queue_trees(force=True)
                    while tree_q:
                        tree_q.pop(0)()
                    a_count = (a_count + 1) // 2 * 2  # keep pairs forming
                elif tree_q:
                    tree_q.pop(0)()

                if c in TAIL_SLICES:
                    tail_stage1(*TAIL_SLICES[c])

            # final chunk's tree + last tail slice (cols 1920:2048)
            queue_trees(force=True)
            while tree_q:
                tree_q.pop(0)()
            tail_stage1(4, 1920, 2048)
            tail_stage2(4, 1920, 2048)

    nc.compile()
    return nc


def _get_program():
    global _PROGRAM
    if _PROGRAM is None:
        _PROGRAM = _build_program()
    return _PROGRAM


def _host_pack(relative_points, W1, b1, W1g, b1g, W2g, b2g):
    X = np.ascontiguousarray(relative_points, dtype=np.float32)
    W1 = np.asarray(W1, np.float32)
    b1 = np.asarray(b1, np.float32)
    W1g = np.asarray(W1g, np.float32)
    b1g = np.asarray(b1g, np.float32)
    W2g = np.asarray(W2g, np.float32)
    b2g = np.asarray(b2g, np.float32)

    wpack = np.zeros((6, 128), np.float32)
    wpack[0:3, 0:64] = W1.T
    wpack[3:6, 64:128] = W1.T
    b1d = np.concatenate([b1, b1]).reshape(128, 1)
    w1gbd = np.zeros((128, 128), np.float32)
    w1gbd[0:64, 0:64] = W1g.T
    w1gbd[64:128, 64:128] = W1g.T
    b1gd = np.concatenate([b1g, b1g]).reshape(128, 1)
    w2gt = np.vstack([W2g.T, W2g.T])  # [128, 128]
    b2gc = np.ascontiguousarray(b2g.reshape(128, 1))

    wpack = wpack.astype(BF16)
    w1gbd = w1gbd.astype(BF16)
    w2gt = np.ascontiguousarray(w2gt.astype(BF16))

    in_maps = []
    for d in range(NCORES):
        Xc = X[d * NPC : (d + 1) * NPC]
        xt6 = np.ascontiguousarray(
            Xc.reshape(G, 2, 512, 3)
            .transpose(1, 3, 0, 2)
            .reshape(6, G * 512)
            .astype(BF16)
        )
        in_maps.append(
            {
                "xt": xt6,
                "wpack": wpack,
                "b1d": b1d,
                "w1gbd": w1gbd,
                "b1gd": b1gd,
                "w2gt": w2gt,
                "b2g": b2gc,
            }
        )
    return in_maps


def _host_unpack(results):
    out = np.empty((S, FG1), np.float32)
    for d in range(NCORES):
        oA = results[d]["outA"].reshape(128, 32, 4, 16)
        oB = results[d]["outB"].reshape(128, 32, 4, 16)
        blk = out[d * SPC : (d + 1) * SPC].reshape(32, 4, 2, 16, 128)
        blk[:, :, 0] = oA.transpose(1, 2, 3, 0)
        blk[:, :, 1] = oB.transpose(1, 2, 3, 0)
    return out


def _numpy_fallback(relative_points, cluster, num_clusters,
                    W1, b1, W1g, b1g, W2g, b2g):
    X = np.asarray(relative_points, np.float32)
    fc1 = np.maximum(X @ np.asarray(W1, np.float32).T + np.asarray(b1, np.float32), 0.0)
    Sn = int(num_clusters)
    cl = np.asarray(cluster).astype(np.int64)
    pooled = np.full((Sn, fc1.shape[1]), -np.inf, np.float32)
    starts = np.flatnonzero(np.r_[True, cl[1:] != cl[:-1]])
    seg_ids = cl[starts]
    pooled[seg_ids] = np.maximum.reduceat(fc1, starts, axis=0)
    h = np.maximum(pooled @ np.asarray(W1g, np.float32).T + np.asarray(b1g, np.float32), 0.0)
    return np.maximum(h @ np.asarray(W2g, np.float32).T + np.asarray(b2g, np.float32), 0.0).astype(np.float32)


def _run_hw(in_maps, trace=False):
    from concourse.bass_utils import run_bass_kernel_spmd

    nc = _get_program()
    return run_bass_kernel_spmd(
        nc, in_maps, list(range(NCORES)), trace=trace
    )


def kernel(relative_points, cluster, num_clusters,
           W1, b1, W1g, b1g, W2g, b2g):
    cl = np.asarray(cluster)
    expected_cl = np.arange(N, dtype=np.int64) // PTS_PER_CLUSTER
    if (
        relative_points.shape != (N, 3)
        or int(num_clusters) != S
        or not np.array_equal(cl, expected_cl)
    ):
        return _numpy_fallback(relative_points, cluster, num_clusters,
                               W1, b1, W1g, b1g, W2g, b2g)

    in_maps = _host_pack(relative_points, W1, b1, W1g, b1g, W2g, b2g)
    res = _run_hw(in_maps, trace=False)
    return _host_unpack(res.results)


def run_traced(inputs):
    """test.py helper: returns (output, exec_time_ns)."""
    in_maps = _host_pack(
        inputs["relative_points"], inputs["W1"], inputs["b1"],
        inputs["W1g"], inputs["b1g"], inputs["W2g"], inputs["b2g"],
    )
    res = _run_hw(in_maps, trace=True)
    return _host_unpack(res.results), res.exec_time_ns
